# revision 5
# baseline (speedup 1.0000x reference)
"""Trainium2 Bass kernel for nn_LogicGatedSNN.

Computation (per output row o, in_features i):
    w      = (synapse_states > 50) as f32            [O, I]
    cur    = w @ x                                   [O]
    v      = 0.8*membrane_potential + cur            [O]
    spk    = (v >= adaptive_threshold) as f32        [O]
    trace' = clip(0.7*eligibility_trace + 3*outer(spk, x), 0, 10)   [O, I]
    thr'   = clip(adaptive_threshold + 0.1*spk - 0.005, 0.5, 10)    [O]
    v'     = v * (1 - spk) * 0.2                     [O]
returns (spk, v', trace', thr')

Sharding: out_features split across 8 NeuronCores (1024 rows each);
x broadcast. No collectives.

Per core: 8 row-tiles of [128, I]; each split into column chunks.
Key fused ops:
  - scalar_tensor_tensor(out, in0=S, scalar=50, in1=xb, op0=is_gt,
    op1=mult, accum_out=cur): binarize + mask-mult + row-sum in ONE
    DVE pass (the whole matvec).
  - scalar_tensor_tensor(out=T, in0=T, scalar=0.7, in1=outer,
    op0=mult, op1=add): trace decay + potentiation in one pass.
"""

import sys

if "/opt/trn_rl_repo" not in sys.path:
    sys.path.insert(0, "/opt/trn_rl_repo")

import numpy as np

import concourse.bass as bass
import concourse.bacc as bacc
import concourse.tile as tile
from concourse import mybir
from concourse.bass_utils import run_bass_kernel_spmd

DT = mybir.dt.float32
Alu = mybir.AluOpType
P = 128
N_CORES = 8
O_FULL = 8192
I_FULL = 8192
THRESH = 50.0


def build_nc(o_sh=O_FULL // N_CORES, i_full=I_FULL, chunk=2048):
    """Build the per-core Bass module. o_sh rows of the output are
    processed by this core; x is the full input vector."""
    nrt = o_sh // P          # row tiles
    nch = i_full // chunk    # column chunks per row tile

    nc = bacc.Bacc(None, target_bir_lowering=False)

    x_d = nc.dram_tensor("x", [i_full], DT, kind="ExternalInput")
    syn_d = nc.dram_tensor("syn", [o_sh, i_full], DT, kind="ExternalInput")
    tr_d = nc.dram_tensor("tr", [o_sh, i_full], DT, kind="ExternalInput")
    mp_d = nc.dram_tensor("mp", [o_sh], DT, kind="ExternalInput")
    th_d = nc.dram_tensor("th", [o_sh], DT, kind="ExternalInput")

    spk_o = nc.dram_tensor("spk_o", [o_sh], DT, kind="ExternalOutput")
    v_o = nc.dram_tensor("v_o", [o_sh], DT, kind="ExternalOutput")
    tro_d = nc.dram_tensor("tr_o", [o_sh, i_full], DT, kind="ExternalOutput")
    th_o = nc.dram_tensor("th_o", [o_sh], DT, kind="ExternalOutput")

    # [o_sh] vectors viewed as [P, nrt]: element (p, n) = row n*P + p
    mp_v = mp_d[:].rearrange("(n p) -> p n", p=P)
    th_v = th_d[:].rearrange("(n p) -> p n", p=P)
    spk_v = spk_o[:].rearrange("(n p) -> p n", p=P)
    v_v = v_o[:].rearrange("(n p) -> p n", p=P)
    tho_v = th_o[:].rearrange("(n p) -> p n", p=P)

    with tile.TileContext(nc) as tc:
        with (
            tc.tile_pool(name="xbp", bufs=1) as xbp,
            tc.tile_pool(name="sp", bufs=3) as sp,
            tc.tile_pool(name="tp", bufs=3) as tp,
            tc.tile_pool(name="outp", bufs=2) as outp,
            tc.tile_pool(name="prp", bufs=2) as prp,
            tc.tile_pool(name="smalls", bufs=1) as smalls,
            tc.tile_pool(name="cols", bufs=4) as cols,
        ):
            # broadcast x to all partitions (one-time DMA)
            xb = xbp.tile([P, i_full], DT)
            nc.sync.dma_start(out=xb[:, :], in_=x_d[None, :].partition_broadcast(P))

            mp_t = smalls.tile([P, nrt], DT)
            nc.sync.dma_start(out=mp_t[:, :], in_=mp_v)
            th_t = smalls.tile([P, nrt], DT)
            nc.sync.dma_start(out=th_t[:, :], in_=th_v)

            spk_t = smalls.tile([P, nrt], DT)
            vo_t = smalls.tile([P, nrt], DT)
            tho_t = smalls.tile([P, nrt], DT)

            for it in range(nrt):
                rows = slice(it * P, (it + 1) * P)
                # ---- matvec: cur[p] = sum_i (S[p,i] > 50) * x[i] ----
                cur = cols.tile([P, nch], DT, tag="cur")
                for c in range(nch):
                    csl = slice(c * chunk, (c + 1) * chunk)
                    s_t = sp.tile([P, chunk], DT, tag="s")
                    nc.sync.dma_start(out=s_t[:, :], in_=syn_d[rows, csl])
                    pr = prp.tile([P, chunk], DT, tag="pr")
                    nc.vector.scalar_tensor_tensor(
                        out=pr[:, :], in0=s_t[:, :], scalar=THRESH,
                        in1=xb[:, csl], op0=Alu.is_gt, op1=Alu.mult,
                        accum_out=cur[:, c:c + 1],
                    )
                # ---- small per-row ops ----
                # v = 0.8*mp + sum_c cur[:, c]
                v_c = cols.tile([P, 1], DT, tag="v")
                nc.vector.tensor_scalar(
                    out=v_c[:, :], in0=mp_t[:, it:it + 1], scalar1=0.8,
                    scalar2=cur[:, 0:1], op0=Alu.mult, op1=Alu.add,
                )
                for c in range(1, nch):
                    nc.vector.tensor_scalar(
                        out=v_c[:, :], in0=v_c[:, :], scalar1=cur[:, c:c + 1],
                        scalar2=None, op0=Alu.add,
                    )
                # spk = (v >= thr)
                nc.vector.tensor_scalar(
                    out=spk_t[:, it:it + 1], in0=v_c[:, :],
                    scalar1=th_t[:, it:it + 1], scalar2=None, op0=Alu.is_ge,
                )
                # spk3 = 3*spk (scale of the outer product)
                spk3 = cols.tile([P, 1], DT, tag="spk3")
                nc.vector.tensor_scalar(
                    out=spk3[:, :], in0=spk_t[:, it:it + 1], scalar1=3.0,
                    scalar2=None, op0=Alu.mult,
                )
                # v' = v * (1-spk) * 0.2  ==  v * (spk*-0.2 + 0.2)
                g = cols.tile([P, 1], DT, tag="g")
                nc.vector.tensor_scalar(
                    out=g[:, :], in0=spk_t[:, it:it + 1], scalar1=-0.2,
                    scalar2=0.2, op0=Alu.mult, op1=Alu.add,
                )
                nc.vector.tensor_tensor(
                    out=vo_t[:, it:it + 1], in0=v_c[:, :], in1=g[:, :],
                    op=Alu.mult,
                )
                # thr' = clip(thr + 0.1*spk - 0.005, 0.5, 10)
                t1 = cols.tile([P, 1], DT, tag="t1")
                nc.vector.tensor_scalar(
                    out=t1[:, :], in0=spk_t[:, it:it + 1], scalar1=0.1,
                    scalar2=-0.005, op0=Alu.mult, op1=Alu.add,
                )
                nc.vector.tensor_scalar(
                    out=t1[:, :], in0=t1[:, :], scalar1=th_t[:, it:it + 1],
                    scalar2=None, op0=Alu.add,
                )
                nc.vector.tensor_scalar(
                    out=tho_t[:, it:it + 1], in0=t1[:, :], scalar1=0.5,
                    scalar2=10.0, op0=Alu.max, op1=Alu.min,
                )
                # ---- trace update ----
                for c in range(nch):
                    csl = slice(c * chunk, (c + 1) * chunk)
                    t_t = tp.tile([P, chunk], DT, tag="t")
                    nc.sync.dma_start(out=t_t[:, :], in_=tr_d[rows, csl])
                    o_t = outp.tile([P, chunk], DT, tag="o")
                    # outer = (3*spk)[p] * x[i]
                    nc.vector.tensor_scalar(
                        out=o_t[:, :], in0=xb[:, csl], scalar1=spk3[:, 0:1],
                        scalar2=None, op0=Alu.mult,
                    )
                    # T = 0.7*T + outer
                    nc.vector.scalar_tensor_tensor(
                        out=t_t[:, :], in0=t_t[:, :], scalar=0.7,
                        in1=o_t[:, :], op0=Alu.mult, op1=Alu.add,
                    )
                    # T = clip(T, 0, 10)
                    nc.vector.tensor_scalar(
                        out=t_t[:, :], in0=t_t[:, :], scalar1=0.0,
                        scalar2=10.0, op0=Alu.max, op1=Alu.min,
                    )
                    nc.sync.dma_start(out=tro_d[rows, csl], in_=t_t[:, :])

            nc.sync.dma_start(out=spk_v, in_=spk_t[:, :])
            nc.sync.dma_start(out=v_v, in_=vo_t[:, :])
            nc.sync.dma_start(out=tho_v, in_=tho_t[:, :])

    nc.compile()
    return nc


_NC_CACHE = {}


def kernel(spike_input, synapse_states, membrane_potential,
           adaptive_threshold, eligibility_trace, _trace=False, _tmpdir=None):
    o_sh = O_FULL // N_CORES
    key = "full"
    if key not in _NC_CACHE:
        _NC_CACHE[key] = build_nc(o_sh=o_sh, i_full=I_FULL)
    nc = _NC_CACHE[key]

    x = np.ascontiguousarray(spike_input, dtype=np.float32)
    syn = np.ascontiguousarray(synapse_states, dtype=np.float32)
    tr = np.ascontiguousarray(eligibility_trace, dtype=np.float32)
    mp = np.ascontiguousarray(membrane_potential, dtype=np.float32)
    th = np.ascontiguousarray(adaptive_threshold, dtype=np.float32)

    in_maps = []
    for i in range(N_CORES):
        r = slice(i * o_sh, (i + 1) * o_sh)
        in_maps.append({
            "x": x,
            "syn": np.ascontiguousarray(syn[r]),
            "tr": np.ascontiguousarray(tr[r]),
            "mp": np.ascontiguousarray(mp[r]),
            "th": np.ascontiguousarray(th[r]),
        })

    br = run_bass_kernel_spmd(nc, in_maps, core_ids=list(range(N_CORES)),
                              trace=_trace, tmpdir=_tmpdir)
    res = br.results

    spikes = np.concatenate([res[i]["spk_o"] for i in range(N_CORES)])
    new_v = np.concatenate([res[i]["v_o"] for i in range(N_CORES)])
    new_tr = np.concatenate([res[i]["tr_o"] for i in range(N_CORES)], axis=0)
    new_th = np.concatenate([res[i]["th_o"] for i in range(N_CORES)])

    out = (spikes.astype(np.float32), new_v.astype(np.float32),
           new_tr.astype(np.float32), new_th.astype(np.float32))
    if _trace:
        return out, br
    return out
